# revision 1
# baseline (speedup 1.0000x reference)
"""minGRU stacked-layer kernel for Trainium2, data-parallel over batch on 8 cores.

Problem: B=8, S=4096, D=512, L=4 minGRU layers, vocab V=32000, C=8 classes.
  h = emb[x]                                  # [B,S,D]
  per layer: z = sigmoid(h@Wz+bz); ht = h@Wh+bh
             h_t = (1-z_t) h_{t-1} + z_t ht_t     (scan over t, h_0 = 0)
  out = h[:, -1] @ Wo + bo                    # [B,C]

Per-core layout (1 sequence per core): activations kept feature-on-partition,
time-on-free ("[d, t]"), so every layer matmul is  W.T @ h  with W's natural
[d, e] layout as lhsT -- no transposes between layers.  The recurrence runs as
the native DVE tensor_tensor_scan along the free axis, chained across 512-wide
time chunks.  Engine split per chunk/e-tile:
  PE : zlin = Wz.T@h, hlin = Wh.T@h  (float32r: full-rate fp32 matmul at
       N=512; weights/emb are host-pre-rounded to the fp32r bit format so
       byte-moving DMAs produce valid operands)
  ACT: a = sigmoid(-(zlin+bz)) = 1-z;  z = sigmoid(zlin+bz)   (fused bias)
  DVE: b = (hlin + bh) * z   (scalar_tensor_tensor, one PSUM operand)
       h = tensor_tensor_scan(a, b, op0=mult, op1=add)  -- fp32 state,
       chained across chunks via initial = prev chunk's last column
Embedding gather: indirect DMA of 128 rows at a time -> [t, d] tiles, then
PE-transpose (128x128 blocks, paired into [128,256] PSUM tiles) + ACT copy
into [d, t] f32r tiles.  PSUM pools coexist (transpose 2 + zlin 3 + hlin 3
banks) so the prologue overlaps layer-0 compute; per-layer weight DMAs are
dep-throttled behind the previous layer's first scan to keep prologue HBM
bandwidth for the gathers.
"""

import os
import sys
import types

import numpy as np

B, S, D, L, V, C = 8, 4096, 512, 4, 32000, 8
P = 128            # SBUF partitions
ED = D // P        # 4 feature tiles
TC = 512           # time-chunk (matmul N / scan length per instruction)
NCH = S // TC      # 8 time chunks
GPC = TC // P      # 4 gather-groups (128 tokens) per time chunk
NG = S // P        # 32 gather groups total

_cache = {}


def _install_ntff_hook_shim():
    """Best-effort: register the axon NTFF profiling hook so trace=True works.

    Harmless if anything is missing -- tracing degrades gracefully."""
    try:
        if "antenv.axon_hooks" in sys.modules:
            return
        import antenv
        from trn_agent_boot.trn_boot import _ntff_profile_via_ctypes

        mod = types.ModuleType("antenv.axon_hooks")
        _h = [None]
        mod.set_axon_ntff_profile_hook = lambda h: _h.__setitem__(0, h)
        mod.get_axon_ntff_profile_hook = lambda: _h[0]
        so = "/opt/axon/libaxon_pjrt.so"
        if os.path.exists(so):
            hook = _ntff_profile_via_ctypes(so)
            if hook is not None:
                mod.set_axon_ntff_profile_hook(hook)
        sys.modules["antenv.axon_hooks"] = mod
        antenv.axon_hooks = mod
    except Exception:
        pass


def _build_nc():
    import concourse.mybir as mybir
    import concourse.tile as tile
    from concourse import bacc
    from concourse.bass import IndirectOffsetOnAxis
    from concourse.masks import make_identity
    from concourse.tile import add_dep_helper

    f32 = mybir.dt.float32
    f32r = mybir.dt.float32r
    i32 = mybir.dt.int32
    AF = mybir.ActivationFunctionType
    OP = mybir.AluOpType

    nc = bacc.Bacc("TRN2", target_bir_lowering=False)

    # emb/Wz/Wh/Wo are fed host-pre-rounded to the fp32r format (fp32 with
    # 11-bit mantissa, low 12 bits zero) so plain byte-moving DMAs produce
    # valid fp32r operands for the full-rate fp32r matmuls.
    x_col = nc.dram_tensor("x_col", [P, NG], i32, kind="ExternalInput")
    emb_d = nc.dram_tensor("emb", [V, D], f32r, kind="ExternalInput")
    wz_d = nc.dram_tensor("Wz", [L, D, D], f32r, kind="ExternalInput")
    wh_d = nc.dram_tensor("Wh", [L, D, D], f32r, kind="ExternalInput")
    bz_d = nc.dram_tensor("bz_t", [P, L * ED], f32, kind="ExternalInput")
    bh_d = nc.dram_tensor("bh_t", [P, L * ED], f32, kind="ExternalInput")
    wo_d = nc.dram_tensor("Wo", [D, C], f32r, kind="ExternalInput")
    bo_d = nc.dram_tensor("bo", [1, C], f32, kind="ExternalInput")
    y_d = nc.dram_tensor("y", [1, C], f32, kind="ExternalOutput")

    with tile.TileContext(nc) as tc:
        with (
            tc.tile_pool(name="const", bufs=1) as cpool,
            tc.tile_pool(name="h", bufs=20) as hpool,
            tc.tile_pool(name="w", bufs=64) as wpool,
            tc.tile_pool(name="acts", bufs=6) as apool,
            tc.tile_pool(name="emb", bufs=4) as epool,
        ):
            ids = cpool.tile([P, NG], i32, name="ids", tag="ids")
            nc.sync.dma_start(ids[:], x_col[:])
            ident = cpool.tile([P, P], f32, name="ident", tag="ident")
            make_identity(nc, ident[:])
            identr = cpool.tile([P, P], f32r, name="identr", tag="identr")
            nc.vector.tensor_copy(identr[:], ident[:])
            bz_sb = cpool.tile([P, L * ED], f32, name="bz_sb", tag="bz")
            nc.sync.dma_start(bz_sb[:], bz_d[:])
            bh_sb = cpool.tile([P, L * ED], f32, name="bh_sb", tag="bh")
            nc.sync.dma_start(bh_sb[:], bh_d[:])
            nbz_sb = cpool.tile([P, L * ED], f32, name="nbz_sb", tag="nbz")
            nc.vector.tensor_scalar_mul(nbz_sb[:], bz_sb[:], -1.0)

            # ---- prologue: embedding gather + transpose into [d, t] tiles
            # PSUM pools coexist (transpose 2 + zlin 3 + hlin 3 = 8 banks) so
            # the prologue interleaves with layer-0 compute.
            h_tiles = [[None] * ED for _ in range(NCH)]
            for c in range(NCH):
                for d in range(ED):
                    h_tiles[c][d] = hpool.tile(
                        [P, TC], f32r, name=f"h0_{c}_{d}", tag="h", bufs=40
                    )
            with (
                tc.tile_pool(name="tp", bufs=2, space="PSUM") as tpp,
                tc.tile_pool(name="zlin", bufs=3, space="PSUM") as zpp,
                tc.tile_pool(name="hlin", bufs=3, space="PSUM") as hpp,
            ):
                for gp in range(0, NG, 2):
                    ets = []
                    for g in (gp, gp + 1):
                        et = epool.tile([P, D], f32r, name=f"et_{g}", tag="e", bufs=4)
                        nc.gpsimd.indirect_dma_start(
                            out=et[:],
                            out_offset=None,
                            in_=emb_d[:],
                            in_offset=IndirectOffsetOnAxis(
                                ap=ids[:, g : g + 1], axis=0
                            ),
                        )
                        ets.append(et)
                    c, j = divmod(gp, GPC)
                    for d in range(ED):
                        pt = tpp.tile([P, 2 * P], f32r, name=f"pt_{gp}_{d}", tag="tp")
                        for i in (0, 1):
                            nc.tensor.transpose(
                                pt[:, i * P : (i + 1) * P],
                                ets[i][:, d * P : (d + 1) * P],
                                identr[:],
                            )
                        dst = h_tiles[c][d][:, j * P : (j + 2) * P]
                        nc.scalar.copy(dst, pt[:])

                # ---- layers
                layer_first_scan = {}
                for l in range(L):
                    # one big DMA per weight matrix, [p, (k e)] layout; layer
                    # l >= 1 loads are gated on layer l-1's first scan so the
                    # prologue's gather DMAs get the HBM bandwidth first.
                    wz_big = wpool.tile(
                        [P, ED, D], f32r, name=f"wzb_{l}", tag="w", bufs=4
                    )
                    dz = nc.sync.dma_start(
                        wz_big[:], wz_d[l].rearrange("(k p) e -> p k e", p=P)
                    )
                    wh_big = wpool.tile(
                        [P, ED, D], f32r, name=f"whb_{l}", tag="w", bufs=4
                    )
                    dh = nc.sync.dma_start(
                        wh_big[:], wh_d[l].rearrange("(k p) e -> p k e", p=P)
                    )
                    if l >= 1 and (l - 1) in layer_first_scan:
                        add_dep_helper(dz.ins, layer_first_scan[l - 1].ins,
                                       reason="throttle weight prefetch")
                        add_dep_helper(dh.ins, layer_first_scan[l - 1].ins,
                                       reason="throttle weight prefetch")
                    wz_t = [
                        [wz_big[:, k, e * P : (e + 1) * P] for e in range(ED)]
                        for k in range(ED)
                    ]
                    wh_t = [
                        [wh_big[:, k, e * P : (e + 1) * P] for e in range(ED)]
                        for k in range(ED)
                    ]

                    new_h = [[None] * ED for _ in range(NCH)]
                    for c in range(NCH):
                        zps = []
                        for e in range(ED):
                            zp = zpp.tile([P, TC], f32, name=f"zp_{l}_{c}_{e}", tag="z")
                            for k in range(ED):
                                nc.tensor.matmul(
                                    zp[:],
                                    wz_t[k][e],
                                    h_tiles[c][k][:],
                                    start=(k == 0),
                                    stop=(k == ED - 1),
                                )
                            zps.append(zp)
                        hps = []
                        for e in range(ED):
                            hp = hpp.tile([P, TC], f32, name=f"hp_{l}_{c}_{e}", tag="hl")
                            for k in range(ED):
                                nc.tensor.matmul(
                                    hp[:],
                                    wh_t[k][e],
                                    h_tiles[c][k][:],
                                    start=(k == 0),
                                    stop=(k == ED - 1),
                                )
                            hps.append(hp)
                        for e in range(ED):
                            le = l * ED + e
                            a_t = apool.tile(
                                [P, TC], f32, name=f"a_{l}_{c}_{e}", tag="a", bufs=8
                            )
                            # a = sigmoid(-(zlin + bz)) = 1 - z
                            nc.scalar.activation(
                                a_t[:],
                                zps[e][:],
                                AF.Sigmoid,
                                bias=nbz_sb[:, le : le + 1],
                                scale=-1.0,
                            )
                            z_t = apool.tile(
                                [P, TC], f32, name=f"z_{l}_{c}_{e}", tag="zt", bufs=8
                            )
                            nc.scalar.activation(
                                z_t[:],
                                zps[e][:],
                                AF.Sigmoid,
                                bias=bz_sb[:, le : le + 1],
                                scale=1.0,
                            )
                            b_t = apool.tile(
                                [P, TC], f32, name=f"b_{l}_{c}_{e}", tag="bt", bufs=8
                            )
                            # b = (hlin + bh) * z, straight from PSUM
                            nc.vector.scalar_tensor_tensor(
                                b_t[:],
                                in0=hps[e][:],
                                scalar=bh_sb[:, le : le + 1],
                                in1=z_t[:],
                                op0=OP.add,
                                op1=OP.mult,
                            )
                            hn = hpool.tile(
                                [P, TC], f32r, name=f"h_{l}_{c}_{e}", tag="h", bufs=40
                            )
                            init = (
                                0.0
                                if c == 0
                                else new_h[c - 1][e][:, TC - 1 : TC].bitcast(f32)
                            )
                            # state = (a * state) + b
                            sc_inst = nc.vector.tensor_tensor_scan(
                                hn[:],
                                a_t[:],
                                b_t[:],
                                init,
                                op0=OP.mult,
                                op1=OP.add,
                            )
                            if l not in layer_first_scan:
                                layer_first_scan[l] = sc_inst
                            new_h[c][e] = hn
                    h_tiles = new_h

            # ---- classifier head on the last timestep
            with tc.tile_pool(name="head", bufs=1, space="PSUM") as hdp:
                wo_t = []
                for k in range(ED):
                    wt = cpool.tile([P, C], f32r, name=f"wo_{k}", tag=f"wo{k}")
                    nc.sync.dma_start(wt[:], wo_d[k * P : (k + 1) * P, :])
                    wo_t.append(wt)
                bo_sb = cpool.tile([1, C], f32, name="bo_sb", tag="bo")
                nc.sync.dma_start(bo_sb[:], bo_d[:])
                op_ps = hdp.tile([1, C], f32, name="op_ps", tag="o")
                for k in range(ED):
                    nc.tensor.matmul(
                        op_ps[:],
                        h_tiles[NCH - 1][k][:, TC - 1 : TC],
                        wo_t[k][:],
                        start=(k == 0),
                        stop=(k == ED - 1),
                    )
                out_sb = cpool.tile([1, C], f32, name="out_sb", tag="y")
                nc.vector.tensor_add(out_sb[:], op_ps[:], bo_sb[:])
                nc.sync.dma_start(y_d[:], out_sb[:])

    nc.compile()
    return nc


def _round_f32r(a):
    """Round fp32 to the fp32r format: 11-bit mantissa (low 12 bits zero),
    round-to-nearest-even.  The result is still a valid fp32 bit pattern."""
    u = np.ascontiguousarray(np.asarray(a, dtype=np.float32)).view(np.uint32).copy()
    u += 0x7FF + ((u >> 12) & 1)
    u &= np.uint32(0xFFFFF000)
    return u.view(np.float32)


def kernel(x, emb, Wz, bz, Wh, bh, Wo, bo):
    _install_ntff_hook_shim()
    from concourse.bass_utils import run_bass_kernel_spmd

    if "nc" not in _cache:
        _cache["nc"] = _build_nc()
    nc = _cache["nc"]

    x = np.asarray(x)
    emb = _round_f32r(emb)
    Wz = _round_f32r(Wz)
    Wh = _round_f32r(Wh)
    Wo = _round_f32r(Wo)
    # bias [L, D] -> [P, L*ED] with (p, l*ED+e) = b[l, e*P+p]
    bz_t = np.ascontiguousarray(
        np.asarray(bz, dtype=np.float32).reshape(L, ED, P).transpose(2, 0, 1).reshape(P, L * ED)
    )
    bh_t = np.ascontiguousarray(
        np.asarray(bh, dtype=np.float32).reshape(L, ED, P).transpose(2, 0, 1).reshape(P, L * ED)
    )
    bo_r = np.ascontiguousarray(np.asarray(bo, dtype=np.float32).reshape(1, C))

    in_maps = []
    for i in range(B):
        # ids column-major: (p, g) = x[i, g*P + p]
        xc = np.ascontiguousarray(x[i].reshape(NG, P).T.astype(np.int32))
        in_maps.append(
            {
                "x_col": xc,
                "emb": emb,
                "Wz": Wz,
                "Wh": Wh,
                "bz_t": bz_t,
                "bh_t": bh_t,
                "Wo": Wo,
                "bo": bo_r,
            }
        )

    res = run_bass_kernel_spmd(nc, in_maps, core_ids=list(range(B)))
    _cache["last_results"] = res
    out = np.stack([res.results[i]["y"][0] for i in range(B)]).astype(np.float32)
    return out

